# revision 9
# baseline (speedup 1.0000x reference)
"""GNN unpool (gather by clique id + scatter-add by node id) on 8 trn2 cores.

Problem: inputs [B=16, C*NC], node_ids/clique_ids [M], output [B, N*C] where
  pooled = inputs.reshape(B, C, NC)
  out[b, c, node_ids[m]] += pooled[b, c, clique_ids[m]]  for each m

Sharding: 8 node ranges x full batch. Core r handles nodes
[6250r, 6250(r+1)) for all 16 batches (bc = 1024 rows). Per-core membership
entries ~= M/8 = 12500, which halves the SWDGE dma_gather index count vs the
old 2x4 sharding: Q7 descriptor generation (~7.5ns/index + ~0.7us/call) was
the pacing engine at 287us.

The host hands every core the same clique-major pooled features in bf16
([NC, 1024]); dma_gather fetches 2KB bf16 token rows per entry.

Entries are packed per core into variable-width node segments (<=128 nodes,
greedy-filled to <=256 entries) so every segment is exactly 2 chunks of 128
gather slots -- no chunk-grid padding (a fixed 128-node segment averages
256 +- 16 entries, straddling the 2-chunk boundary and wasting ~25% slots).
Segment positions are uniform across cores (SPMD): chunk grid is the max
over cores; short cores pad with idx-0 gathers and all-zero one-hot columns.

Scatter is a one-hot matmul: host PRE-COMPUTES the one-hot H[entry, rel]
tables (bf16) and DMAs them in (the old kernel built them on DVE via
is_equal at ~1us each -- 215us of DVE busy, the #2 bottleneck). Per chunk:
PE matmuls H_c.T [128e x 128n] @ tok_c [128e, 512|512] into two psum banks
(bf16 in, fp32 accum). ACT+DVE evacuate psum -> bf16 staging, DMA -> out
[NSEG*128, 1024] bf16 in segment-position order; host drops pad rows,
transposes, casts.
"""

import math
import sys

import numpy as np

sys.path.insert(0, "/opt/trn_rl_repo")

import ml_dtypes  # noqa: E402

from concourse import bacc, bass, mybir, tile  # noqa: E402
from concourse.bass_utils import run_bass_kernel_spmd  # noqa: E402

P = 128
N_CORES = 8
BC = 1024  # full batch x channels = 16*64
SEG_CAP = 2 * P  # max entries per segment (2 gather chunks)
GPOS = 8  # segment positions per gather call
SGRP = 8  # positions per output staging tile


# ---------------------------------------------------------------- host planning


def _plan(node_ids, clique_ids, NC, N):
    node_ids = np.asarray(node_ids).astype(np.int64)
    clique_ids = np.asarray(clique_ids).astype(np.int64)
    M = node_ids.shape[0]
    NR = N // N_CORES  # nodes per core (6250)
    assert NR * N_CORES == N

    rng = node_ids // NR
    enode = node_ids - rng * NR

    # Per-core greedy segmentation: contiguous node windows, <=128 nodes,
    # <=SEG_CAP entries each.
    per_core = []
    nseg_r = []
    for r in range(N_CORES):
        m = rng == r
        en = enode[m]
        cl = clique_ids[m]
        order = np.argsort(en, kind="stable")
        en = en[order]
        cl = cl[order]
        cnt = np.bincount(en, minlength=NR)
        cum = np.concatenate([[0], np.cumsum(cnt)])  # entries before node i
        starts = []
        widths = []
        s = 0
        while s < NR:
            e = int(np.searchsorted(cum, cum[s] + SEG_CAP, side="right")) - 1
            e = min(max(e, s + 1), s + P, NR)
            starts.append(s)
            widths.append(e - s)
            s = e
        starts = np.array(starts, np.int64)
        widths = np.array(widths, np.int64)
        scount = cum[starts + widths] - cum[starts]
        assert scount.max() <= SEG_CAP
        per_core.append(dict(en=en, cl=cl, cum=cum, starts=starts,
                             widths=widths, scount=scount))
        nseg_r.append(len(starts))

    NSEG = max(nseg_r)
    counts = np.zeros((N_CORES, NSEG), np.int64)
    for r in range(N_CORES):
        counts[r, : nseg_r[r]] = per_core[r]["scount"]
    cap = counts.max(axis=0)
    nchunks = np.maximum(1, (cap + P - 1) // P)  # expected: all 2
    seg_base = np.zeros(NSEG + 1, np.int64)
    seg_base[1:] = np.cumsum(nchunks)
    CT = int(seg_base[NSEG])

    idx_tbls = []
    h_tbls = []
    for r in range(N_CORES):
        pc = per_core[r]
        en, cl, cum = pc["en"], pc["cl"], pc["cum"]
        starts, widths = pc["starts"], pc["widths"]
        nsr = nseg_r[r]
        # entry j (node-sorted) -> segment p, slot seg_base[p]*128 + rank
        p_of = np.searchsorted(starts + widths, en, side="right")
        ebase = cum[starts]  # first entry index of each segment
        j = np.arange(len(en))
        slots = seg_base[p_of] * P + (j - ebase[p_of])
        stream = np.zeros(CT * P, np.int16)
        stream[slots] = cl.astype(np.int16)
        wrapped = stream.reshape(-1, 16).T  # [16, CT*8]
        idx_tbls.append(np.tile(wrapped, (8, 1)).copy())  # [128, CT*8]

        h_flat = np.zeros((CT * P, P), np.float32)
        h_flat[slots, en - starts[p_of]] = 1.0
        h_tile = (
            h_flat.reshape(CT, P, P).transpose(1, 0, 2).reshape(P, CT * P)
        )
        h_tbls.append(h_tile.astype(ml_dtypes.bfloat16))

    # Small leading groups shrink the pipeline-fill bubble (PE can start
    # after a 2-position gather instead of an 8-position one); tapered tail
    # groups shrink the drain (last gather's transfer+PE+evac+write chain).
    front = [2, 2, 4]
    tail = [4, 3, 2]
    mid = NSEG - sum(front) - sum(tail)
    if mid >= 0:
        sizes = front + [GPOS] * (mid // GPOS)
        if mid % GPOS:
            sizes.append(mid % GPOS)
        sizes += tail
    else:
        sizes = []
        rem = NSEG
        while rem > 0:
            sizes.append(min(4, rem))
            rem -= sizes[-1]
    groups = []
    p0 = 0
    for sz in sizes:
        groups.append((p0, p0 + sz))
        p0 += sz
    assert p0 == NSEG

    return dict(
        M=M,
        NC=NC,
        N=N,
        NR=NR,
        NSEG=NSEG,
        nseg_r=nseg_r,
        per_core=per_core,
        nchunks=nchunks,
        seg_base=seg_base,
        CT=CT,
        idx_tbls=idx_tbls,
        h_tbls=h_tbls,
        groups=groups,
    )


# ---------------------------------------------------------------- device build


def _build(plan):
    NSEG = plan["NSEG"]
    seg_base = plan["seg_base"]
    CT = plan["CT"]
    groups = plan["groups"]

    f32 = mybir.dt.float32
    bf16 = mybir.dt.bfloat16
    i16 = mybir.dt.int16

    NCKG = max(int(seg_base[p1] - seg_base[p0]) for p0, p1 in groups)

    nc = bacc.Bacc(None, target_bir_lowering=False)

    poolT_d = nc.dram_tensor("pooledT", [plan["NC"], BC], bf16,
                             kind="ExternalInput")
    idx_d = nc.dram_tensor("idxtbl", [P, CT * 8], i16, kind="ExternalInput")
    h_d = nc.dram_tensor("htbl", [P, CT * P], bf16, kind="ExternalInput")
    out_d = nc.dram_tensor("out", [NSEG * P, BC], bf16, kind="ExternalOutput")

    with tile.TileContext(nc) as tc:
        with (
            tc.tile_pool(name="const", bufs=1) as constp,
            tc.tile_pool(name="tok", bufs=4) as tokp,
            tc.tile_pool(name="opsum", bufs=4, space="PSUM") as opsum,
            tc.tile_pool(name="stage", bufs=2) as stagep,
        ):
            f32r = mybir.dt.float32r

            idx_t = constp.tile([P, CT * 8], i16)
            nc.sync.dma_start(idx_t[:], idx_d[:])

            # The whole one-hot table rides in during the ~17us head (gather
            # ucode IRAM load + engine init) when DMA bandwidth is otherwise
            # idle -- keeps H bytes out of the drain-paced steady state.
            h_t = constp.tile([P, CT * P], bf16)
            nc.sync.dma_start(h_t[:], h_d[:])

            # Warm-up gather with an on-chip zero index tile: absorbs the
            # ~6us gather-ucode IRAM load while the real idx table is still
            # in flight. All-zero idxs fetch row 0 into a scratch tile.
            widx = constp.tile([P, 8], i16)
            nc.vector.memset(widx[:], 0)
            wtok = constp.tile([P, 1, BC], bf16)
            nc.gpsimd.dma_gather(
                out_ap=wtok[:, :, :].bitcast(f32r),
                in_ap=poolT_d[:].bitcast(f32r),
                idxs_ap=widx[:],
                num_idxs=P,
                num_idxs_reg=P,
                elem_size=BC // 2,
                single_packet=False,
            )

            # Issue all gathers up-front on the gpsimd queue: Q7 descriptor
            # generation is the serial pacer and must never idle. Pool-buffer
            # reuse (bufs=3) throttles them against PE consumption. The
            # f32r bitcast (2048B rows as 512x4B) keeps the ucode on the
            # 4-byte path: the 2-byte path generates descriptors ~25% slower
            # (9.3 vs 7.5 ns/idx measured).
            tok_tiles = []
            for p0, p1 in groups:
                c0, c1 = int(seg_base[p0]), int(seg_base[p1])
                nck = c1 - c0
                ut = tokp.tile([P, NCKG, BC], bf16, tag="tok")
                nc.gpsimd.dma_gather(
                    out_ap=ut[:, :nck, :].bitcast(f32r),
                    in_ap=poolT_d[:].bitcast(f32r),
                    idxs_ap=idx_t[:, c0 * 8 : c1 * 8],
                    num_idxs=nck * P,
                    num_idxs_reg=nck * P,
                    elem_size=BC // 2,
                    single_packet=False,
                )
                tok_tiles.append(ut)

            cur_stage = None
            p_st = 0
            for gi, (p0, p1) in enumerate(groups):
                c0 = int(seg_base[p0])
                ut = tok_tiles[gi]
                for p in range(p0, p1):
                    ca, cb = int(seg_base[p]), int(seg_base[p + 1])
                    # one 2-bank psum tile per position; the two matmul
                    # series land in its bank-aligned halves, and a single
                    # instruction evacuates both (DVE per-op overhead is
                    # ~0.4-0.7us, so fewer, larger ops win)
                    pq = opsum.tile([P, BC], f32, tag="ops")
                    for c in range(ca, cb):
                        lc = c - c0
                        hs = h_t[:, c * P : (c + 1) * P]
                        nc.tensor.matmul(
                            out=pq[:, : BC // 2],
                            lhsT=hs,
                            rhs=ut[:, lc, : BC // 2],
                            start=(c == ca),
                            stop=(c == cb - 1),
                        )
                        nc.tensor.matmul(
                            out=pq[:, BC // 2 :],
                            lhsT=hs,
                            rhs=ut[:, lc, BC // 2 :],
                            start=(c == ca),
                            stop=(c == cb - 1),
                        )
                    if cur_stage is None:
                        cur_stage = stagep.tile([P, SGRP, BC], bf16, tag="st")
                        p_st = p
                    if p % 2 == 0:
                        nc.scalar.copy(cur_stage[:, p - p_st, :], pq[:])
                    else:
                        nc.vector.tensor_copy(cur_stage[:, p - p_st, :], pq[:])
                    if p - p_st + 1 == SGRP or p == NSEG - 1:
                        nb = p - p_st + 1
                        nc.sync.dma_start(
                            out_d[p_st * P : (p + 1) * P, :].rearrange(
                                "(t r) c -> r t c", t=nb
                            ),
                            cur_stage[:, :nb, :],
                        )
                        cur_stage = None

    nc.finalize()
    return nc


# ---------------------------------------------------------------- entry points

_CACHE = {}


def _get_program(inputs):
    inputs_arr = np.asarray(inputs["inputs"])
    node_ids = np.asarray(inputs["node_ids"])
    clique_ids = np.asarray(inputs["clique_ids"])
    N = int(inputs["nodes"])
    C = int(inputs["n_channels"])
    B, units_dim = inputs_arr.shape
    NC = units_dim // C

    key = (
        B,
        C,
        NC,
        N,
        node_ids.shape[0],
        hash(node_ids.tobytes()),
        hash(clique_ids.tobytes()),
    )
    if key not in _CACHE:
        plan = _plan(node_ids, clique_ids, NC, N)
        nc = _build(plan)
        _CACHE[key] = (plan, nc)
    return _CACHE[key]


def _run(inputs, trace=False):
    inputs_arr = np.asarray(inputs["inputs"]).astype(np.float32)
    N = int(inputs["nodes"])
    C = int(inputs["n_channels"])
    B = inputs_arr.shape[0]
    NC = inputs_arr.shape[1] // C

    plan, nc = _get_program(inputs)
    NR = plan["NR"]
    nseg_r = plan["nseg_r"]

    # clique-major bf16 pooled features, shared by every core
    poolT = np.ascontiguousarray(
        inputs_arr.reshape(B * C, NC).T
    ).astype(ml_dtypes.bfloat16)

    in_maps = []
    for r in range(N_CORES):
        in_maps.append(
            {
                "pooledT": poolT,
                "idxtbl": plan["idx_tbls"][r],
                "htbl": plan["h_tbls"][r],
            }
        )

    res = run_bass_kernel_spmd(
        nc, in_maps, core_ids=list(range(N_CORES)), trace=trace
    )

    out = np.empty((B, C, N), np.float32)
    for r in range(N_CORES):
        pc = plan["per_core"][r]
        starts, widths = pc["starts"], pc["widths"]
        o = np.asarray(res.results[r]["out"]).astype(np.float32)
        blocks = o.reshape(plan["NSEG"], P, B * C)
        full = np.concatenate(
            [blocks[p, : widths[p]] for p in range(nseg_r[r])], axis=0
        )  # [NR, 1024] in node order
        out[:, :, r * NR : (r + 1) * NR] = full.T.reshape(B, C, NR)
    return out.reshape(B, C * N), res


def kernel(**inputs) -> np.ndarray:
    out, _ = _run(inputs, trace=False)
    return out
